# revision 30
# baseline (speedup 1.0000x reference)
"""Conv2D 3x3 (NCHW, OIHW, stride 1, pad 1) on 8 Trainium2 NeuronCores.

Problem shape: input (32, 128, 56, 56) fp32, weights (256, 128, 3, 3) fp32,
output (32, 256, 56, 56) fp32.

Strategy — width-axis Winograd F(2,3), 1.5x fewer PE columns than the
direct 9-tap conv (the PE matmul stream is the bottleneck engine):
  - Data-parallel over batch: 4 images per core, weights replicated.
  - Host applies the 1D Winograd F(2,3) input transform along W to the
    zero-padded image: for each padded row r (58 rows) and tile t (28
    2-wide output tiles), V0=d[2t]-d[2t+2], V1=d[2t+1]+d[2t+2],
    V2=d[2t+2]-d[2t+1], V3=d[2t+1]-d[2t+3], stored as 4 fp16 planes of
    [ci, 58*28].  Weights become U[dy,p] = G @ w-taps (G the F(2,3)
    weight transform), fp16.
  - Device: per image, co-half h, and 14-row chunk c, accumulate
        m_p[co, 392] = sum_dy U[h,dy,p][ci,co].T @ V_p[ci, rows 14c+dy]
    (12 matmuls, free dim 392, contract 128) into one PSUM bank per p.
    All chunks slice one 8-bank PSUM tile (parity-rotated bank groups)
    so dependency tracking is per bank and the PE never stalls on whole
    tile recycling.
  - Output transform fused into the PSUM drain and spread over three
    engines: ScalarE copies m1,m2 to SBUF bf16, GPSIMD adds s=m1+m2,
    VectorE computes Y0 = s+m0 and Y1 = (m1-m2)-m3, writing fp16 output
    planes that DMA out.  Host interleaves the two w-phase planes and
    upcasts to fp32.
  - DMA: inputs ride the scalar HWDGE queue, weights+outputs the sync
    queue (a single queue saturates); images are prefetched one ahead;
    14 dummy matmuls bridge the HAM clock-ramp window at the start.

Measured on hw: 82.9us vs 114.2us for the direct 9-tap fp16 baseline;
rel err 2.5e-3 (gate 2e-2).
"""

import sys

sys.path.insert(0, "/opt/trn_rl_repo")

import numpy as np

N_CORES = 8
N_FULL = 32
IMGS = N_FULL // N_CORES  # images per core
CIN = 128
COUT = 256
H = W = 56
HP = 58  # padded rows
T = 28  # winograd tiles per row (2 output cols each)
NP = 4  # winograd positions per tile
VROW = HP * T  # 1624 elements per V plane
ROWS_PER_CHUNK = 14
N_CHUNKS = H // ROWS_PER_CHUNK  # 4
FD = ROWS_PER_CHUNK * T  # 392 moving elements per matmul
PIX = H * W  # 3136

_CACHE = {}


def _split_sync_waits(nc, mybir, max_waits=1):
    """The walrus build in this container rejects instructions carrying
    more than one semaphore wait; hoist extras onto preceding NOPs on the
    same engine (engine executes them in order, semantics preserved)."""
    ctr = 0
    for f in nc.m.functions:
        for bb in f.blocks:
            new_insts = []
            for ins in bb.instructions:
                si = getattr(ins, "sync_info", None)
                if si is not None and si.on_wait and len(si.on_wait) > max_waits:
                    waits = list(si.on_wait)
                    extra, keep = waits[:-max_waits], waits[-max_waits:]
                    for i in range(0, len(extra), max_waits):
                        ctr += 1
                        nop = mybir.InstNoOp(
                            name=f"{ins.name}_wsplit{ctr}",
                            engine=ins.engine,
                            sync_info=mybir.SyncInfo(
                                on_wait=extra[i : i + max_waits], on_update=[]
                            ),
                            bass_nofuse=True,
                        )
                        new_insts.append(nop)
                    si.on_wait = keep
                new_insts.append(ins)
            bb.instructions[:] = new_insts
    return ctr


# input V-plane row ranges per DMA piece (lead piece first so chunk 0
# can start as early as possible).  Pieces matter even for prefetched
# images: each piece's write-after-read wait covers only its own row
# range of the previous image in the buffer, so transfers start as those
# rows retire instead of after the full image.
DMA_ROWS_FIRST = ((0, 16), (16, 30), (30, 44), (44, 58))
DMA_ROWS_PREFETCH = ((0, 30), (30, 58))


def _build():
    import concourse.bass as bass
    import concourse.mybir as mybir
    import concourse.tile as tile

    f32 = mybir.dt.float32
    f16 = mybir.dt.float16
    bf16 = mybir.dt.bfloat16

    nc = bass.Bass()
    x = nc.declare_dram_parameter("x", [IMGS, CIN, NP * VROW], f16, isOutput=False)
    w = nc.declare_dram_parameter("w", [CIN, 2 * 3 * NP * 128], f16, isOutput=False)
    out = nc.declare_dram_parameter("out", [IMGS, COUT, 2 * PIX // 2], f16, isOutput=True)

    x4 = x.rearrange("n p (v q) -> n p v q", v=NP)  # q = 1624 (row*28)
    w5 = w.rearrange("p (h y v c) -> p h y v c", h=2, y=3, v=NP)
    out4 = out.rearrange("n c (v q) -> n c v q", v=2)  # q = 1568 (row*28)

    with tile.TileContext(nc) as tc:
        with (
            tc.tile_pool(name="wpool", bufs=1) as wpool,
            tc.tile_pool(name="xpool", bufs=2) as xpool,
            tc.tile_pool(name="cpool", bufs=4) as cpool,
            tc.tile_pool(name="spool", bufs=4) as spool,
            tc.tile_pool(name="opool", bufs=4) as opool,
            tc.tile_pool(name="psum", bufs=1, space="PSUM") as pspool,
        ):
            # One 8-bank PSUM tile, manually rotated: chunk parity q uses
            # banks 4q..4q+3 (one per winograd position p).  Slicing a single
            # tile gives per-bank dependency tracking, so the next chunk's
            # matmuls only wait for the reader of the specific bank they
            # write, not for the whole 4-bank group (tile-pool rotation
            # stalled the PE ~640ns per chunk).
            psa = pspool.tile([128, 8, 512], f32, name="psa")

            # PE warmup: dummy matmuls while the first DMAs are in flight so
            # HAM un-throttles (1.2->2.4 GHz) before the real matmuls start.
            # 14 x N=256 cold matmuls (~213ns each) bridge the gap until the
            # first real chunk's operands have landed -- an idle gap between
            # warmup and the real stream lets the free-running HAM activity
            # window re-arm and keeps the PE at 1.2 GHz for ~10us (measured).
            warm = wpool.tile([128, 256], f16, name="warm")
            nc.vector.memzero(warm[:])
            for _ in range(14):
                nc.tensor.matmul(
                    psa[:, 7, 0:256], lhsT=warm[:, 0:128], rhs=warm[:],
                    start=True, stop=True,
                )

            # DMA ring split: weights + outputs ride the sync HWDGE queue,
            # input planes ride the scalar queue -- one queue for all 13MB
            # saturates and the final output DMAs drain ~3us late.
            wt = wpool.tile([CIN, 2 * 3 * NP * 128], f16)
            wt5 = wt.rearrange("p (h y v c) -> p h y v c", h=2, y=3, v=NP)
            nc.sync.dma_start(out=wt5[:, 0], in_=w5[:, 0])
            nc.sync.dma_start(out=wt5[:, 1], in_=w5[:, 1])

            def load_image(n):
                vt = xpool.tile([CIN, NP, VROW], f16)
                vt3 = vt.rearrange("p v (r t) -> p v r t", t=T)
                xr = x4[n].rearrange("p v (r t) -> p v r t", t=T)
                rows = DMA_ROWS_FIRST if n == 0 else DMA_ROWS_PREFETCH
                for r0, r1 in rows:
                    nc.scalar.dma_start(out=vt3[:, :, r0:r1, :], in_=xr[:, :, r0:r1, :])
                return vt

            vts = {0: load_image(0)}
            chunk_idx = 0
            for n in range(IMGS):
                # prefetch next image first so its DMAs issue (and stream)
                # while this image computes
                if n + 1 < IMGS:
                    vts[n + 1] = load_image(n + 1)
                vt = vts.pop(n)
                for h in range(2):
                    for c in range(N_CHUNKS):
                        q = 4 * (chunk_idx % 2)
                        chunk_idx += 1
                        ps = psa[:, q : q + NP, :]
                        # p order (1,2,0,3): the ScalarE copies of m1/m2 can
                        # start while the p0/p3 matmuls still run.
                        for p in (1, 2, 0, 3):
                            for dy in range(3):
                                row0 = c * ROWS_PER_CHUNK + dy
                                nc.tensor.matmul(
                                    ps[:, p, 0:FD],
                                    lhsT=wt5[:, h, dy, p, :],
                                    rhs=vt[:, p, row0 * T : row0 * T + FD],
                                    start=(dy == 0),
                                    stop=(dy == 2),
                                )
                        c1 = cpool.tile([128, FD], bf16, name="c1")
                        c2 = cpool.tile([128, FD], bf16, name="c2")
                        nc.scalar.copy(out=c1[:], in_=ps[:, 1, 0:FD])
                        nc.scalar.copy(out=c2[:], in_=ps[:, 2, 0:FD])
                        s = spool.tile([128, FD], bf16, name="s")
                        d = spool.tile([128, FD], bf16, name="d")
                        ot = opool.tile([128, 2, FD], f16, name="ot")
                        # s = m1+m2 on the (otherwise idle) GPSIMD engine --
                        # both inputs are SBUF 16-bit which GPSIMD can reach;
                        # keeps the DVE for the PSUM-reading ops.
                        nc.gpsimd.tensor_add(s[:], c1[:], c2[:])
                        nc.vector.tensor_add(ot[:, 0, :], s[:], ps[:, 0, 0:FD])
                        nc.vector.tensor_sub(d[:], c1[:], c2[:])
                        nc.vector.tensor_sub(ot[:, 1, :], d[:], ps[:, 3, 0:FD])
                        cs = slice(c * FD, (c + 1) * FD)
                        hs = slice(h * 128, (h + 1) * 128)
                        nc.sync.dma_start(out=out4[n, hs, :, cs], in_=ot[:, :, :])

    _split_sync_waits(nc, mybir)
    return nc


def _prep_inputs(input_batch, weights):
    x = np.asarray(input_batch, dtype=np.float32)
    wf = np.asarray(weights, dtype=np.float32)
    xp = np.zeros((N_FULL, CIN, HP, HP), np.float32)
    xp[:, :, 1:-1, 1:-1] = x
    V = np.empty((N_FULL, CIN, NP, HP, T), np.float32)
    V[:, :, 0] = xp[:, :, :, 0:56:2] - xp[:, :, :, 2:58:2]
    V[:, :, 1] = xp[:, :, :, 1:57:2] + xp[:, :, :, 2:58:2]
    V[:, :, 2] = xp[:, :, :, 2:58:2] - xp[:, :, :, 1:57:2]
    V[:, :, 3] = xp[:, :, :, 1:57:2] - xp[:, :, :, 3:58:2]
    V16 = V.reshape(N_FULL, CIN, NP * VROW).astype(np.float16)

    g0, g1, g2 = wf[..., 0], wf[..., 1], wf[..., 2]  # (COUT, CIN, 3[dy])
    U = np.stack([g0, (g0 + g1 + g2) / 2, (g0 - g1 + g2) / 2, g2], axis=-1)
    # U: (COUT, CIN, dy, p) -> layout [ci, h, dy, p, c]
    wt = np.ascontiguousarray(
        U.reshape(2, 128, CIN, 3, NP)
        .transpose(2, 0, 3, 4, 1)
        .reshape(CIN, 2 * 3 * NP * 128)
        .astype(np.float16)
    )
    in_maps = []
    for i in range(N_CORES):
        in_maps.append(
            {"x": np.ascontiguousarray(V16[i * IMGS : (i + 1) * IMGS]), "w": wt}
        )
    return in_maps


def _postprocess(raw):
    # raw: (IMGS, COUT, 2*1568) f16, plane-major -> (IMGS, COUT, 56, 56) f32
    y = raw.reshape(IMGS, COUT, 2, H, T).transpose(0, 1, 3, 4, 2)
    return y.reshape(IMGS, COUT, H, W).astype(np.float32)


def _run(input_batch, weights, trace=False):
    from concourse.bass_utils import run_bass_kernel_spmd

    if "nc" not in _CACHE:
        _CACHE["nc"] = _build()
    nc = _CACHE["nc"]
    in_maps = _prep_inputs(np.asarray(input_batch), np.asarray(weights))
    res = run_bass_kernel_spmd(nc, in_maps, list(range(N_CORES)), trace=trace)
    outs = [_postprocess(res.results[i]["out"]) for i in range(N_CORES)]
    full = np.concatenate(outs, axis=0)
    return full, res


def kernel(input_batch, weights):
    full, _ = _run(input_batch, weights, trace=False)
    return full


# revision 31
# speedup vs baseline: 1.0412x; 1.0412x over previous
"""Conv2D 3x3 (NCHW, OIHW, stride 1, pad 1) on 8 Trainium2 NeuronCores.

Problem shape: input (32, 128, 56, 56) fp32, weights (256, 128, 3, 3) fp32,
output (32, 256, 56, 56) fp32.

Strategy — width-axis Winograd F(2,3), 1.5x fewer PE columns than the
direct 9-tap conv (the PE matmul stream is the bottleneck engine):
  - Data-parallel over batch: 4 images per core, weights replicated.
  - Host applies the 1D Winograd F(2,3) input transform along W to the
    zero-padded image: for each padded row r (58 rows) and tile t (28
    2-wide output tiles), V0=d[2t]-d[2t+2], V1=d[2t+1]+d[2t+2],
    V2=d[2t+2]-d[2t+1], V3=d[2t+1]-d[2t+3], stored as 4 fp16 planes of
    [ci, 58*28].  Weights become U[dy,p] = G @ w-taps (G the F(2,3)
    weight transform), fp16.
  - Device: per image, co-half h, and 14-row chunk c, accumulate
        m_p[co, 392] = sum_dy U[h,dy,p][ci,co].T @ V_p[ci, rows 14c+dy]
    (12 matmuls, free dim 392, contract 128) into one PSUM bank per p.
    All chunks slice one 8-bank PSUM tile (parity-rotated bank groups)
    so dependency tracking is per bank and the PE never stalls on whole
    tile recycling.
  - Output transform fused into the PSUM drain and spread over three
    engines: ScalarE copies m1,m2 to SBUF bf16, GPSIMD adds s=m1+m2,
    VectorE computes Y0 = s+m0 and Y1 = (m1-m2)-m3, writing fp16 output
    planes that DMA out.  Host interleaves the two w-phase planes and
    upcasts to fp32.
  - DMA: inputs ride the scalar HWDGE queue, weights+outputs the sync
    queue (a single queue saturates); images are prefetched one ahead;
    14 dummy matmuls bridge the HAM clock-ramp window at the start.

Measured on hw: 82.9us vs 114.2us for the direct 9-tap fp16 baseline;
rel err 2.5e-3 (gate 2e-2).
"""

import sys

sys.path.insert(0, "/opt/trn_rl_repo")

import numpy as np

N_CORES = 8
N_FULL = 32
IMGS = N_FULL // N_CORES  # images per core
CIN = 128
COUT = 256
H = W = 56
HP = 58  # padded rows
T = 28  # winograd tiles per row (2 output cols each)
NP = 4  # winograd positions per tile
VROW = HP * T  # 1624 elements per V plane
ROWS_PER_CHUNK = 14
N_CHUNKS = H // ROWS_PER_CHUNK  # 4
FD = ROWS_PER_CHUNK * T  # 392 moving elements per matmul
PIX = H * W  # 3136

_CACHE = {}


def _split_sync_waits(nc, mybir, max_waits=1):
    """The walrus build in this container rejects instructions carrying
    more than one semaphore wait; hoist extras onto preceding NOPs on the
    same engine (engine executes them in order, semantics preserved)."""
    ctr = 0
    for f in nc.m.functions:
        for bb in f.blocks:
            new_insts = []
            for ins in bb.instructions:
                si = getattr(ins, "sync_info", None)
                if si is not None and si.on_wait and len(si.on_wait) > max_waits:
                    waits = list(si.on_wait)
                    extra, keep = waits[:-max_waits], waits[-max_waits:]
                    for i in range(0, len(extra), max_waits):
                        ctr += 1
                        nop = mybir.InstNoOp(
                            name=f"{ins.name}_wsplit{ctr}",
                            engine=ins.engine,
                            sync_info=mybir.SyncInfo(
                                on_wait=extra[i : i + max_waits], on_update=[]
                            ),
                            bass_nofuse=True,
                        )
                        new_insts.append(nop)
                    si.on_wait = keep
                new_insts.append(ins)
            bb.instructions[:] = new_insts
    return ctr


# input V-plane row ranges per DMA piece (lead piece first so chunk 0
# can start as early as possible).  Pieces matter even for prefetched
# images: each piece's write-after-read wait covers only its own row
# range of the previous image in the buffer, so transfers start as those
# rows retire instead of after the full image.
DMA_ROWS_FIRST = ((0, 16), (16, 30), (30, 44), (44, 58))
DMA_ROWS_PREFETCH = ((0, 30), (30, 58))


def _build():
    import concourse.bass as bass
    import concourse.mybir as mybir
    import concourse.tile as tile

    f32 = mybir.dt.float32
    f16 = mybir.dt.float16
    bf16 = mybir.dt.bfloat16

    nc = bass.Bass()
    x = nc.declare_dram_parameter("x", [IMGS, CIN, NP * VROW], f16, isOutput=False)
    w = nc.declare_dram_parameter("w", [CIN, 2 * 3 * NP * 128], f16, isOutput=False)
    out = nc.declare_dram_parameter("out", [IMGS, COUT, 2 * PIX // 2], f16, isOutput=True)

    x4 = x.rearrange("n p (v q) -> n p v q", v=NP)  # q = 1624 (row*28)
    w5 = w.rearrange("p (h y v c) -> p h y v c", h=2, y=3, v=NP)
    out4 = out.rearrange("n c (v q) -> n c v q", v=2)  # q = 1568 (row*28)

    with tile.TileContext(nc) as tc:
        with (
            tc.tile_pool(name="wpool", bufs=1) as wpool,
            tc.tile_pool(name="xpool", bufs=2) as xpool,
            tc.tile_pool(name="cpool", bufs=4) as cpool,
            tc.tile_pool(name="spool", bufs=4) as spool,
            tc.tile_pool(name="opool", bufs=4) as opool,
            tc.tile_pool(name="psum", bufs=1, space="PSUM") as pspool,
        ):
            # One 8-bank PSUM tile, manually rotated: chunk parity q uses
            # banks 4q..4q+3 (one per winograd position p).  Slicing a single
            # tile gives per-bank dependency tracking, so the next chunk's
            # matmuls only wait for the reader of the specific bank they
            # write, not for the whole 4-bank group (tile-pool rotation
            # stalled the PE ~640ns per chunk).
            psa = pspool.tile([128, 8, 512], f32, name="psa")

            # PE warmup: dummy matmuls while the first DMAs are in flight so
            # HAM un-throttles (1.2->2.4 GHz) before the real matmuls start.
            # 14 x N=256 cold matmuls (~213ns each) bridge the gap until the
            # first real chunk's operands have landed -- an idle gap between
            # warmup and the real stream lets the free-running HAM activity
            # window re-arm and keeps the PE at 1.2 GHz for ~10us (measured).
            warm = wpool.tile([128, 256], f16, name="warm")
            nc.vector.memzero(warm[:])
            for _ in range(20):
                nc.tensor.matmul(
                    psa[:, 7, 0:256], lhsT=warm[:, 0:128], rhs=warm[:],
                    start=True, stop=True,
                )

            # DMA ring split: weights + outputs ride the sync HWDGE queue,
            # input planes ride the scalar queue -- one queue for all 13MB
            # saturates and the final output DMAs drain ~3us late.
            wt = wpool.tile([CIN, 2 * 3 * NP * 128], f16)
            wt5 = wt.rearrange("p (h y v c) -> p h y v c", h=2, y=3, v=NP)
            nc.sync.dma_start(out=wt5[:, 0], in_=w5[:, 0])
            nc.sync.dma_start(out=wt5[:, 1], in_=w5[:, 1])

            def load_image(n):
                vt = xpool.tile([CIN, NP, VROW], f16)
                vt3 = vt.rearrange("p v (r t) -> p v r t", t=T)
                xr = x4[n].rearrange("p v (r t) -> p v r t", t=T)
                rows = DMA_ROWS_FIRST if n == 0 else DMA_ROWS_PREFETCH
                for r0, r1 in rows:
                    nc.scalar.dma_start(out=vt3[:, :, r0:r1, :], in_=xr[:, :, r0:r1, :])
                return vt

            vts = {0: load_image(0)}
            chunk_idx = 0
            for n in range(IMGS):
                # prefetch next image first so its DMAs issue (and stream)
                # while this image computes
                if n + 1 < IMGS:
                    vts[n + 1] = load_image(n + 1)
                vt = vts.pop(n)
                for h in range(2):
                    for c in range(N_CHUNKS):
                        q = 4 * (chunk_idx % 2)
                        chunk_idx += 1
                        ps = psa[:, q : q + NP, :]
                        # p order (1,2,0,3): the ScalarE copies of m1/m2 can
                        # start while the p0/p3 matmuls still run.
                        for p in (1, 2, 0, 3):
                            for dy in range(3):
                                row0 = c * ROWS_PER_CHUNK + dy
                                nc.tensor.matmul(
                                    ps[:, p, 0:FD],
                                    lhsT=wt5[:, h, dy, p, :],
                                    rhs=vt[:, p, row0 * T : row0 * T + FD],
                                    start=(dy == 0),
                                    stop=(dy == 2),
                                )
                        c1 = cpool.tile([128, FD], bf16, name="c1")
                        c2 = cpool.tile([128, FD], bf16, name="c2")
                        nc.scalar.copy(out=c1[:], in_=ps[:, 1, 0:FD])
                        nc.scalar.copy(out=c2[:], in_=ps[:, 2, 0:FD])
                        s = spool.tile([128, FD], bf16, name="s")
                        d = spool.tile([128, FD], bf16, name="d")
                        ot = opool.tile([128, 2, FD], f16, name="ot")
                        # s = m1+m2 on the (otherwise idle) GPSIMD engine --
                        # both inputs are SBUF 16-bit which GPSIMD can reach;
                        # keeps the DVE for the PSUM-reading ops.
                        nc.gpsimd.tensor_add(s[:], c1[:], c2[:])
                        nc.vector.tensor_add(ot[:, 0, :], s[:], ps[:, 0, 0:FD])
                        nc.vector.tensor_sub(d[:], c1[:], c2[:])
                        nc.vector.tensor_sub(ot[:, 1, :], d[:], ps[:, 3, 0:FD])
                        cs = slice(c * FD, (c + 1) * FD)
                        hs = slice(h * 128, (h + 1) * 128)
                        nc.sync.dma_start(out=out4[n, hs, :, cs], in_=ot[:, :, :])

    _split_sync_waits(nc, mybir)
    return nc


def _prep_inputs(input_batch, weights):
    x = np.asarray(input_batch, dtype=np.float32)
    wf = np.asarray(weights, dtype=np.float32)
    xp = np.zeros((N_FULL, CIN, HP, HP), np.float32)
    xp[:, :, 1:-1, 1:-1] = x
    V = np.empty((N_FULL, CIN, NP, HP, T), np.float32)
    V[:, :, 0] = xp[:, :, :, 0:56:2] - xp[:, :, :, 2:58:2]
    V[:, :, 1] = xp[:, :, :, 1:57:2] + xp[:, :, :, 2:58:2]
    V[:, :, 2] = xp[:, :, :, 2:58:2] - xp[:, :, :, 1:57:2]
    V[:, :, 3] = xp[:, :, :, 1:57:2] - xp[:, :, :, 3:58:2]
    V16 = V.reshape(N_FULL, CIN, NP * VROW).astype(np.float16)

    g0, g1, g2 = wf[..., 0], wf[..., 1], wf[..., 2]  # (COUT, CIN, 3[dy])
    U = np.stack([g0, (g0 + g1 + g2) / 2, (g0 - g1 + g2) / 2, g2], axis=-1)
    # U: (COUT, CIN, dy, p) -> layout [ci, h, dy, p, c]
    wt = np.ascontiguousarray(
        U.reshape(2, 128, CIN, 3, NP)
        .transpose(2, 0, 3, 4, 1)
        .reshape(CIN, 2 * 3 * NP * 128)
        .astype(np.float16)
    )
    in_maps = []
    for i in range(N_CORES):
        in_maps.append(
            {"x": np.ascontiguousarray(V16[i * IMGS : (i + 1) * IMGS]), "w": wt}
        )
    return in_maps


def _postprocess(raw):
    # raw: (IMGS, COUT, 2*1568) f16, plane-major -> (IMGS, COUT, 56, 56) f32
    y = raw.reshape(IMGS, COUT, 2, H, T).transpose(0, 1, 3, 4, 2)
    return y.reshape(IMGS, COUT, H, W).astype(np.float32)


def _run(input_batch, weights, trace=False):
    from concourse.bass_utils import run_bass_kernel_spmd

    if "nc" not in _CACHE:
        _CACHE["nc"] = _build()
    nc = _CACHE["nc"]
    in_maps = _prep_inputs(np.asarray(input_batch), np.asarray(weights))
    res = run_bass_kernel_spmd(nc, in_maps, list(range(N_CORES)), trace=trace)
    outs = [_postprocess(res.results[i]["out"]) for i in range(N_CORES)]
    full = np.concatenate(outs, axis=0)
    return full, res


def kernel(input_batch, weights):
    full, _ = _run(input_batch, weights, trace=False)
    return full
